# revision 19
# baseline (speedup 1.0000x reference)
"""Trainium2 Bass kernel for AttentionBlock (B=8, C=256, L=2048), data-parallel
over batch across 8 NeuronCores.

Math (one batch per core, x: [C, L]):
    scores^T = x^T M x + (u.x) 1^T   with  M = Wq^T Wk,  u = Wk^T bq / sqrt(C)
    pT = exp(scores^T / sqrt(C) + ux)        [m, l], m on partitions
    denom = 16.ones^T pT   (PE DoubleRow matmuls over the fp8 pT tiles)
    ctx = vT^T pT,  vT = x^T Wv^T
    out = ctx * (1/denom16) + (x + bv)

All big matmuls run fp8e4m3 DoubleRow (~216ns per 512-col matmul at steady
clock = 2x bf16). Host scales M/Wv by 16 and u by 256 for fp8 range; the 16x
on v cancels through the denominator and the 256x on u at the ux eviction.

exp: each chunk's scores land in TWO separate PSUM tiles - sA (cols 0:1024)
consumed by ScalarE ACTIVATE Exp, sB (cols 1024:2048) consumed by the DVE via
a Schraudolph bit-trick (one tensor_scalar emitting int8(scores*a+b) whose
int8 bits ARE fp8e4m3 exp). Separate tiles let the two engines run truly
concurrently (two consumers of one tile get serialized by the framework).

The per-key exp bias u.x is computed directly in column layout on the PE
(16 one-column DoubleRow matmuls, lhsT = the x8 key slice), replacing the
DRAM-bounce transpose. DMA: weights ride one coalesced blob; x8 splits
across four engine queues (per-queue DMA is ~35 GB/s); the bf16 residual
trails on sync/scalar.
"""

import numpy as np
import ml_dtypes

import concourse.bass as bass
import concourse.tile as tile
from concourse import bacc, mybir
from concourse.bass_utils import run_bass_kernel_spmd

B, C, L = 8, 256, 2048
P = 128                 # partitions
NMC = L // P            # 16 m-chunks (key blocks)
NB = 512                # matmul moving free dim (one PSUM bank)
NLN = L // NB           # 4 col slices of 512
SCALE = float(C) ** -0.5
WARMUP_MMS = 5

LN2 = float(np.log(2.0))
EXP_A = 8.0 / (256.0 * LN2)          # scores_psum -> fp8 bits slope
EXP_C = 56.0 + 0.5                   # fp8e4m3 exponent-bias magic + tweak
UXB_A = 8.0 / LN2                    # ux -> bits bias slope

F32 = mybir.dt.float32
BF16 = mybir.dt.bfloat16
FP8 = mybir.dt.float8e4
U8 = mybir.dt.uint8
I8 = mybir.dt.int8
DR = mybir.MatmulPerfMode.DoubleRow
MUL = mybir.AluOpType.mult
ADD = mybir.AluOpType.add

WBLOB = 1064            # 512 mt8 + 512 wvt8 + 32 u8 + 8 bv, bytes/partition

_COMPILED = None


def build_nc():
    nc = bacc.Bacc("TRN2", target_bir_lowering=False, debug=False, num_devices=8)

    xbf_d = nc.dram_tensor("xbf", [P, 2, L], BF16, kind="ExternalInput").ap()
    x8_d = nc.dram_tensor("x8", [P, 2, L], FP8, kind="ExternalInput").ap()
    wb_d = nc.dram_tensor("wb", [P, WBLOB], U8, kind="ExternalInput").ap()
    out_d = nc.dram_tensor("out", [C, L], F32, kind="ExternalOutput").ap()

    with tile.TileContext(nc) as tc:
        with (
            tc.tile_pool(name="const", bufs=1) as const,
            tc.tile_pool(name="data", bufs=1) as data,
            tc.tile_pool(name="evict", bufs=4) as evict,
        ):
            # ---- constants ----
            c16 = const.tile([P, 2, NB], FP8)       # warmup + denominator lhsT
            nc.vector.memset(c16[:], 16.0)          # keep gpsimd queue clear

            wb = const.tile([P, WBLOB], U8, tag="wb")
            mt8 = wb[:, 0:512].bitcast(FP8).rearrange("p (a c) -> p a c", a=2)
            wvt8 = wb[:, 512:1024].bitcast(FP8).rearrange("p (a c) -> p a c", a=2)
            u8 = wb[:, 1024:1056].bitcast(FP8).rearrange("p (a c) -> p a c", a=2)
            bv_sb = wb[:, 1056:1064].bitcast(F32).rearrange("p (a c) -> p a c", a=2)

            x8 = data.tile([P, 2, L], FP8, tag="x8", name="x8")
            x_bf = data.tile([P, 2, L], BF16, tag="xbf", name="xbf")

            # DMA is packet-rate-bound (~25-50ns/packet, one packet per
            # contiguous per-partition run): ship x8 as four 128-packet
            # transfers across all three queues, low columns first so the
            # first projections start early; weights blob on sync first
            nc.sync.dma_start(out=wb[:], in_=wb_d[:])
            nc.gpsimd.dma_start(out=x8[:, 0, 0:1024], in_=x8_d[:, 0, 0:1024])
            nc.scalar.dma_start(out=x8[:, 1, 0:1024], in_=x8_d[:, 1, 0:1024])
            nc.gpsimd.dma_start(out=x8[:, 0, 1024:2048], in_=x8_d[:, 0, 1024:2048])
            nc.sync.dma_start(out=x8[:, 1, 1024:2048], in_=x8_d[:, 1, 1024:2048])
            nc.sync.dma_start(out=x_bf[:, 0, :], in_=xbf_d[:, 0, :])

            w8 = data.tile([P, 2, L], FP8, tag="w8", name="w8")
            vT8 = data.tile([P, NMC, C], FP8, tag="vT8")
            pT8 = data.tile([P, NMC, L], FP8, tag="pT8")
            recip = data.tile([P, L], F32, tag="recip")
            ux_col = data.tile([P, NMC, 1], F32, tag="uxcol")
            uxb_col = data.tile([P, NMC, 1], F32, tag="uxbcol")
            junk = data.tile([P, 16], BF16, tag="junk")

            # warm the exp table while DMAs land; second residual half rides
            # the scalar queue behind it
            nc.scalar.activation(out=junk[:], in_=c16[:, 0, 0:16],
                                 func=mybir.ActivationFunctionType.Exp)
            nc.scalar.dma_start(out=x_bf[:, 1, :], in_=xbf_d[:, 1, :])

            # ---- phase 1: projections ----
            with tc.tile_pool(name="psA", bufs=1, space=bass.MemorySpace.PSUM) as psA:
                warm = psA.tile([P, NB], F32, tag="warm", name="warm", bufs=1)
                for _ in range(WARMUP_MMS):
                    nc.tensor.matmul(warm[:], c16[:, 0:2, 0:P],
                                     c16[:, 0:2, 0:NB],
                                     start=True, stop=True, perf_mode=DR)

                # ux in column layout: upc[:, mc] = sum_c u[c] x[c, mc*128+p]
                upc = psA.tile([P, 16], F32, tag="up", name="up", bufs=1)

                def ux_mms(lo, hi):
                    for mc in range(lo, hi):
                        nc.tensor.matmul(
                            upc[:, mc:mc + 1],
                            x8[:, 0:2, mc * P:(mc + 1) * P],
                            u8[:, 0:2, 0:1],
                            start=True, stop=True, perf_mode=DR)

                def w_mms(h):
                    for oc in range(2):
                        wp = psA.tile([P, 1024], F32, tag="big", name="wp", bufs=2)
                        for ln in range(2):
                            c0 = h * 1024 + ln * NB
                            nc.tensor.matmul(
                                wp[:, ln * NB:(ln + 1) * NB],
                                mt8[:, 0:2, oc * P:(oc + 1) * P],
                                x8[:, 0:2, c0:c0 + NB],
                                start=True, stop=True, perf_mode=DR)
                        dst = w8[:, oc, h * 1024:(h + 1) * 1024]
                        if oc == 0:
                            nc.scalar.copy(out=dst, in_=wp[:])
                        else:
                            nc.vector.tensor_copy(out=dst, in_=wp[:])

                ux_mms(0, 8)         # needs only the low x8 columns
                w_mms(0)
                w_mms(1)
                ux_mms(8, 16)
                # evict ux psum -> fp32 column bias + transformed DVE bias
                nc.vector.tensor_scalar_mul(ux_col[:], upc[:], 1.0 / 256.0)
                nc.vector.tensor_scalar(uxb_col[:], ux_col[:], UXB_A, EXP_C,
                                        op0=MUL, op1=ADD)

            # ---- phase 2: scores + concurrent two-engine exp ----
            with tc.tile_pool(name="psS", bufs=1, space=bass.MemorySpace.PSUM) as psS:
                for mc in range(NMC):
                    lhsT = w8[:, 0:2, mc * P:(mc + 1) * P]
                    sa = psS.tile([P, 1024], F32, tag="sa", name="sa", bufs=2)
                    sb = psS.tile([P, 1024], F32, tag="sb", name="sb", bufs=2)
                    for half, st in ((0, sa), (1, sb)):
                        for ln in range(2):
                            col = half * 1024 + ln * NB
                            nc.tensor.matmul(
                                st[:, ln * NB:(ln + 1) * NB],
                                lhsT, x8[:, 0:2, col:col + NB],
                                start=True, stop=True, perf_mode=DR)
                    nc.scalar.activation(
                        out=pT8[:, mc, 0:1024], in_=sa[:],
                        func=mybir.ActivationFunctionType.Exp,
                        scale=1.0 / 256.0, bias=ux_col[:, mc, :])
                    if mc in (5, 10):
                        # rebalance: ScalarE is faster per column than the
                        # DVE, so it takes both halves of two chunks
                        nc.scalar.activation(
                            out=pT8[:, mc, 1024:2048], in_=sb[:],
                            func=mybir.ActivationFunctionType.Exp,
                            scale=1.0 / 256.0, bias=ux_col[:, mc, :])
                    else:
                        nc.vector.tensor_scalar(
                            pT8[:, mc, 1024:2048].bitcast(I8), sb[:],
                            EXP_A, uxb_col[:, mc, :], op0=MUL, op1=ADD)

            # ---- phase 3: vT projection + denominator + context + epilogue ----
            with tc.tile_pool(name="psC", bufs=1, space=bass.MemorySpace.PSUM) as psC:
                ds_t = {}
                ctx_t = {}
                store_eng = [nc.sync, nc.scalar, nc.gpsimd]

                NPAIR = NMC // 2

                # vT[m, c] = sum_c' x[c', m] (16 WvT)[c', c]; group g feeds
                # the ctx/denominator key-pair mp == g
                def vt_mms(g):
                    vp = psC.tile([P, NB], F32, tag="vp", name="vp", bufs=2)
                    for i2 in range(2):
                        mc = g * 2 + i2
                        nc.tensor.matmul(
                            vp[:, i2 * C:(i2 + 1) * C],
                            x8[:, 0:2, mc * P:(mc + 1) * P],
                            wvt8[:, 0:2, 0:C],
                            start=True, stop=True, perf_mode=DR)
                    dst = vT8[:, g * 2:(g + 1) * 2, :]
                    if g % 2 == 0:
                        nc.scalar.copy(out=dst, in_=vp[:])
                    else:
                        nc.vector.tensor_copy(out=dst, in_=vp[:])

                def den_mms(qt, lo, hi):
                    cols = slice(qt * NB, (qt + 1) * NB)
                    if lo == 0:
                        ds_t[qt] = psC.tile([P, NB], F32, tag="d", name="d",
                                            bufs=2)
                    ds = ds_t[qt]
                    for mp in range(lo, hi):
                        nc.tensor.matmul(
                            ds[:], c16[:, 0:2, 0:P],
                            pT8[:, 2 * mp:2 * mp + 2, cols],
                            start=(mp == 0), stop=(mp == NPAIR - 1),
                            perf_mode=DR)
                    if hi == NPAIR:
                        nc.vector.reciprocal_approx_fast(out=recip[:, cols],
                                                         in_=ds[:])

                def ctx_mms(qt, lo, hi):
                    cols = slice(qt * NB, (qt + 1) * NB)
                    for cc in range(2):
                        if lo == 0:
                            ctx_t[(qt, cc)] = psC.tile([P, NB], F32, tag="ctx",
                                                       name="ctx", bufs=4)
                        ct = ctx_t[(qt, cc)]
                        for mp in range(lo, hi):
                            nc.tensor.matmul(
                                ct[:],
                                vT8[:, 2 * mp:2 * mp + 2, cc * P:(cc + 1) * P],
                                pT8[:, 2 * mp:2 * mp + 2, cols],
                                start=(mp == 0), stop=(mp == NPAIR - 1),
                                perf_mode=DR)

                def ctx_evict(qt, nsub):
                    sub = NB // nsub
                    for cc in range(2):
                        rows = slice(cc * P, (cc + 1) * P)
                        for si in range(nsub):
                            c0 = qt * NB + si * sub
                            cols = slice(c0, c0 + sub)
                            pcols = slice(si * sub, (si + 1) * sub)
                            t = evict.tile([P, sub], F32, tag="t", name="t")
                            nc.vector.tensor_mul(t[:], ctx_t[(qt, cc)][:, pcols],
                                                 recip[:, cols])
                            o = evict.tile([P, sub], F32, tag="o", name="o")
                            # bv is folded into x_bf host-side; residual-add
                            # on GpSimd (idle here), alternating with the DVE
                            # on the last quarter for a fast drain
                            eng_o = nc.gpsimd if (qt < NLN - 1 or si % 2 == 0) \
                                else nc.vector
                            eng_o.tensor_add(o[:], x_bf[:, cc, cols], t[:])
                            eng = store_eng[(qt * 2 + cc + si) % 3]
                            eng.dma_start(out=out_d[rows, cols], in_=o[:])

                # weave: vT groups first (their evictions land while the
                # denominator streams), quarter-0 key-pairs 0..6 before pair 7
                # so the PE never stalls on the last chunk's exp
                for g in range(4):
                    vt_mms(g)
                den_mms(0, 0, NPAIR - 1)
                for g in range(4, 8):
                    vt_mms(g)
                ctx_mms(0, 0, NPAIR - 1)
                den_mms(0, NPAIR - 1, NPAIR)
                ctx_mms(0, NPAIR - 1, NPAIR)
                ctx_evict(0, 1)
                for qt in range(1, NLN):
                    den_mms(qt, 0, NPAIR)
                    ctx_mms(qt, 0, NPAIR)
                    ctx_evict(qt, 1 if qt < NLN - 1 else 2)

    nc.compile()
    return nc


def get_compiled():
    global _COMPILED
    if _COMPILED is None:
        _COMPILED = build_nc()
    return _COMPILED


def _shuffle(a):
    """[2*P, N...] -> [P, 2, N...] partition-major."""
    return np.ascontiguousarray(a.reshape(2, P, *a.shape[1:]).transpose(1, 0, *range(2, a.ndim + 1)))


def make_in_maps(inputs):
    x = np.ascontiguousarray(np.asarray(inputs["x"], dtype=np.float32))
    Wq = np.asarray(inputs["Wq"], np.float32)
    Wk = np.asarray(inputs["Wk"], np.float32)
    Wv = np.asarray(inputs["Wv"], np.float32)
    M = Wq.T @ Wk                                   # scores = x^T M x
    u = SCALE * (Wk.T @ np.asarray(inputs["bq"], np.float32))
    u8 = np.zeros((C, 16), ml_dtypes.float8_e4m3)
    u8[:, 0] = (256.0 * u).astype(ml_dtypes.float8_e4m3)
    mt8 = _shuffle(np.ascontiguousarray((16.0 * M).T).astype(ml_dtypes.float8_e4m3))
    wvt8 = _shuffle(np.ascontiguousarray((16.0 * Wv).T).astype(ml_dtypes.float8_e4m3))
    u8s = _shuffle(u8)
    bvs = _shuffle(np.asarray(inputs["bv"], np.float32).reshape(C, 1))
    wb = np.concatenate([
        mt8.reshape(P, 512).view(np.uint8),
        wvt8.reshape(P, 512).view(np.uint8),
        u8s.reshape(P, 32).view(np.uint8),
        np.ascontiguousarray(bvs.reshape(P, 2)).view(np.uint8),
    ], axis=1)
    assert wb.shape == (P, WBLOB)
    bvc = np.asarray(inputs["bv"], np.float32).reshape(C, 1)
    return [{"xbf": _shuffle((x[i] + bvc).astype(ml_dtypes.bfloat16)),
             "x8": _shuffle(x[i].astype(ml_dtypes.float8_e4m3)),
             "wb": wb} for i in range(B)]


def run(inputs, trace=False, **kwargs):
    nc = get_compiled()
    res = run_bass_kernel_spmd(nc, make_in_maps(inputs),
                               core_ids=list(range(B)), trace=trace, **kwargs)
    out = np.stack([res.results[i]["out"] for i in range(B)], axis=0)
    return out.astype(np.float32), res


def kernel(**inputs):
    out, _ = run(inputs)
    return out


# revision 21
# speedup vs baseline: 1.1421x; 1.1421x over previous
"""Trainium2 Bass kernel for AttentionBlock (B=8, C=256, L=2048), data-parallel
over batch across 8 NeuronCores.

Math (one batch per core, x: [C, L]):
    scores^T = x^T M x + (u.x) 1^T   with  M = Wq^T Wk,  u = Wk^T bq / sqrt(C)
    pT = exp(scores^T / sqrt(C) + ux)        [m, l], m on partitions
    denom = 16.ones^T pT   (PE DoubleRow matmuls over the fp8 pT tiles)
    ctx = vT^T pT,  vT = x^T Wv^T
    out = ctx * (1/denom16) + (x + bv)

All big matmuls run fp8e4m3 DoubleRow (~216ns per 512-col matmul at steady
clock = 2x bf16). Host scales M/Wv by 16 and u by 256 for fp8 range; the 16x
on v cancels through the denominator and the 256x on u at the ux eviction.

exp: each chunk's scores land in TWO separate PSUM tiles - sA (cols 0:1024)
consumed by ScalarE ACTIVATE Exp, sB (cols 1024:2048) consumed by the DVE via
a Schraudolph bit-trick (one tensor_scalar emitting int8(scores*a+b) whose
int8 bits ARE fp8e4m3 exp). Separate tiles let the two engines run truly
concurrently (two consumers of one tile get serialized by the framework).

The per-key exp bias u.x is computed directly in column layout on the PE
(16 one-column DoubleRow matmuls, lhsT = the x8 key slice), replacing the
DRAM-bounce transpose. DMA: weights ride one coalesced blob; x8 splits
across four engine queues (per-queue DMA is ~35 GB/s); the bf16 residual
trails on sync/scalar.
"""

import numpy as np
import ml_dtypes

import concourse.bass as bass
import concourse.tile as tile
from concourse import bacc, mybir
from concourse.bass_utils import run_bass_kernel_spmd

B, C, L = 8, 256, 2048
P = 128                 # partitions
NMC = L // P            # 16 m-chunks (key blocks)
NB = 512                # matmul moving free dim (one PSUM bank)
NLN = L // NB           # 4 col slices of 512
SCALE = float(C) ** -0.5
WARMUP_MMS = 5

LN2 = float(np.log(2.0))
EXP_A = 8.0 / (256.0 * LN2)          # scores_psum -> fp8 bits slope
EXP_C = 56.0 + 0.5                   # fp8e4m3 exponent-bias magic + tweak
UXB_A = 8.0 / LN2                    # ux -> bits bias slope

F32 = mybir.dt.float32
BF16 = mybir.dt.bfloat16
FP8 = mybir.dt.float8e4
U8 = mybir.dt.uint8
I8 = mybir.dt.int8
DR = mybir.MatmulPerfMode.DoubleRow
MUL = mybir.AluOpType.mult
ADD = mybir.AluOpType.add

WBLOB = 1064            # 512 mt8 + 512 wvt8 + 32 u8 + 8 bv, bytes/partition

_COMPILED = None


def build_nc():
    nc = bacc.Bacc("TRN2", target_bir_lowering=False, debug=False, num_devices=8)

    xbf_d = nc.dram_tensor("xbf", [P, 2, L], BF16, kind="ExternalInput").ap()
    x8_d = nc.dram_tensor("x8", [P, 2, L], FP8, kind="ExternalInput").ap()
    wb_d = nc.dram_tensor("wb", [P, WBLOB], U8, kind="ExternalInput").ap()
    out_d = nc.dram_tensor("out", [C, L], F32, kind="ExternalOutput").ap()

    with tile.TileContext(nc) as tc:
        with (
            tc.tile_pool(name="const", bufs=1) as const,
            tc.tile_pool(name="data", bufs=1) as data,
            tc.tile_pool(name="evict", bufs=4) as evict,
        ):
            # ---- constants ----
            c16 = const.tile([P, 2, NB], FP8)       # warmup + denominator lhsT
            nc.vector.memset(c16[:], 16.0)          # keep gpsimd queue clear

            wb = const.tile([P, WBLOB], U8, tag="wb")
            mt8 = wb[:, 0:512].bitcast(FP8).rearrange("p (a c) -> p a c", a=2)
            wvt8 = wb[:, 512:1024].bitcast(FP8).rearrange("p (a c) -> p a c", a=2)
            u8 = wb[:, 1024:1056].bitcast(FP8).rearrange("p (a c) -> p a c", a=2)
            bv_sb = wb[:, 1056:1064].bitcast(F32).rearrange("p (a c) -> p a c", a=2)

            x8 = data.tile([P, 2, L], FP8, tag="x8", name="x8")
            x_bf = data.tile([P, 2, L], BF16, tag="xbf", name="xbf")

            # DMA is packet-rate-bound (~25-50ns/packet, one packet per
            # contiguous per-partition run): ship x8 as four 128-packet
            # transfers across all three queues, low columns first so the
            # first projections start early; weights blob on sync first
            nc.sync.dma_start(out=wb[:], in_=wb_d[:])
            nc.gpsimd.dma_start(out=x8[:, 0, 0:1024], in_=x8_d[:, 0, 0:1024])
            nc.scalar.dma_start(out=x8[:, 1, 0:1024], in_=x8_d[:, 1, 0:1024])
            nc.gpsimd.dma_start(out=x8[:, 0, 1024:2048], in_=x8_d[:, 0, 1024:2048])
            nc.sync.dma_start(out=x8[:, 1, 1024:2048], in_=x8_d[:, 1, 1024:2048])
            nc.sync.dma_start(out=x_bf[:, 0, :], in_=xbf_d[:, 0, :])

            w8 = data.tile([P, 2, L], FP8, tag="w8", name="w8")
            vT8 = data.tile([P, NMC, C], FP8, tag="vT8")
            pT8 = data.tile([P, NMC, L], FP8, tag="pT8")
            recip = data.tile([P, L], F32, tag="recip")
            ux_col = data.tile([P, NMC, 1], F32, tag="uxcol")
            uxb_col = data.tile([P, NMC, 1], F32, tag="uxbcol")
            junk = data.tile([P, 16], BF16, tag="junk")

            # warm the exp table while DMAs land; second residual half rides
            # the scalar queue behind it
            nc.scalar.activation(out=junk[:], in_=c16[:, 0, 0:16],
                                 func=mybir.ActivationFunctionType.Exp)
            nc.scalar.dma_start(out=x_bf[:, 1, :], in_=xbf_d[:, 1, :])

            # ---- phase 1: projections ----
            with tc.tile_pool(name="psA", bufs=1, space=bass.MemorySpace.PSUM) as psA:
                warm = psA.tile([P, NB], F32, tag="warm", name="warm", bufs=1)
                for _ in range(WARMUP_MMS):
                    nc.tensor.matmul(warm[:], c16[:, 0:2, 0:P],
                                     c16[:, 0:2, 0:NB],
                                     start=True, stop=True, perf_mode=DR)

                # ux in column layout: upc[:, mc] = sum_c u[c] x[c, mc*128+p]
                upc = psA.tile([P, 16], F32, tag="up", name="up", bufs=1)

                def ux_mms(lo, hi):
                    for mc in range(lo, hi):
                        nc.tensor.matmul(
                            upc[:, mc:mc + 1],
                            x8[:, 0:2, mc * P:(mc + 1) * P],
                            u8[:, 0:2, 0:1],
                            start=True, stop=True, perf_mode=DR)

                def w_mms(h):
                    for oc in range(2):
                        wp = psA.tile([P, 1024], F32, tag="big", name="wp", bufs=2)
                        for ln in range(2):
                            c0 = h * 1024 + ln * NB
                            nc.tensor.matmul(
                                wp[:, ln * NB:(ln + 1) * NB],
                                mt8[:, 0:2, oc * P:(oc + 1) * P],
                                x8[:, 0:2, c0:c0 + NB],
                                start=True, stop=True, perf_mode=DR)
                        dst = w8[:, oc, h * 1024:(h + 1) * 1024]
                        if oc == 0:
                            nc.scalar.copy(out=dst, in_=wp[:])
                        else:
                            nc.vector.tensor_copy(out=dst, in_=wp[:])

                # w-h0 first: its evictions gate the scores phase; ux rides
                # in the gap while the high x8 columns land
                w_mms(0)
                ux_mms(0, 8)
                w_mms(1)
                ux_mms(8, 16)
                # evict ux psum -> fp32 column bias + transformed DVE bias
                nc.vector.tensor_scalar_mul(ux_col[:], upc[:], 1.0 / 256.0)
                nc.vector.tensor_scalar(uxb_col[:], ux_col[:], UXB_A, EXP_C,
                                        op0=MUL, op1=ADD)

                # vT[m, c] = sum_c' x[c', m] (16 WvT)[c', c]; 8 groups of 2
                for g in range(8):
                    vp = psA.tile([P, NB], F32, tag="small", name="vp", bufs=2)
                    for i2 in range(2):
                        mc = g * 2 + i2
                        nc.tensor.matmul(
                            vp[:, i2 * C:(i2 + 1) * C],
                            x8[:, 0:2, mc * P:(mc + 1) * P],
                            wvt8[:, 0:2, 0:C],
                            start=True, stop=True, perf_mode=DR)
                    dst = vT8[:, g * 2:(g + 1) * 2, :]
                    if g % 2 == 0:
                        nc.scalar.copy(out=dst, in_=vp[:])
                    else:
                        nc.vector.tensor_copy(out=dst, in_=vp[:])

            # ---- phase 2: scores + concurrent two-engine exp ----
            with tc.tile_pool(name="psS", bufs=1, space=bass.MemorySpace.PSUM) as psS:
                for mc in range(NMC):
                    lhsT = w8[:, 0:2, mc * P:(mc + 1) * P]
                    sa = psS.tile([P, 1024], F32, tag="sa", name="sa", bufs=2)
                    sb = psS.tile([P, 1024], F32, tag="sb", name="sb", bufs=2)
                    for half, st in ((0, sa), (1, sb)):
                        for ln in range(2):
                            col = half * 1024 + ln * NB
                            nc.tensor.matmul(
                                st[:, ln * NB:(ln + 1) * NB],
                                lhsT, x8[:, 0:2, col:col + NB],
                                start=True, stop=True, perf_mode=DR)
                    nc.scalar.activation(
                        out=pT8[:, mc, 0:1024], in_=sa[:],
                        func=mybir.ActivationFunctionType.Exp,
                        scale=1.0 / 256.0, bias=ux_col[:, mc, :])
                    if mc in (5, 10):
                        # rebalance: ScalarE is faster per column than the
                        # DVE, so it takes both halves of two chunks
                        nc.scalar.activation(
                            out=pT8[:, mc, 1024:2048], in_=sb[:],
                            func=mybir.ActivationFunctionType.Exp,
                            scale=1.0 / 256.0, bias=ux_col[:, mc, :])
                    else:
                        nc.vector.tensor_scalar(
                            pT8[:, mc, 1024:2048].bitcast(I8), sb[:],
                            EXP_A, uxb_col[:, mc, :], op0=MUL, op1=ADD)

            # ---- phase 3: per-quarter denominator + context + epilogue ----
            with tc.tile_pool(name="psC", bufs=1, space=bass.MemorySpace.PSUM) as psC:
                ds = psC.tile([P, L], F32, tag="d", name="d", bufs=1)
                ctx_t = {}
                store_eng = [nc.sync, nc.scalar, nc.gpsimd]

                NPAIR = NMC // 2

                def den_mms(qt, lo, hi):
                    cols = slice(qt * NB, (qt + 1) * NB)
                    for mp in range(lo, hi):
                        nc.tensor.matmul(
                            ds[:, cols], c16[:, 0:2, 0:P],
                            pT8[:, 2 * mp:2 * mp + 2, cols],
                            start=(mp == 0), stop=(mp == NPAIR - 1),
                            perf_mode=DR)
                    if hi == NPAIR:
                        nc.vector.reciprocal_approx_fast(out=recip[:, cols],
                                                         in_=ds[:, cols])

                def ctx_mms(qt, lo, hi):
                    cols = slice(qt * NB, (qt + 1) * NB)
                    for cc in range(2):
                        if lo == 0:
                            ctx_t[(qt, cc)] = psC.tile([P, NB], F32, tag="ctx",
                                                       name="ctx", bufs=4)
                        ct = ctx_t[(qt, cc)]
                        for mp in range(lo, hi):
                            nc.tensor.matmul(
                                ct[:],
                                vT8[:, 2 * mp:2 * mp + 2, cc * P:(cc + 1) * P],
                                pT8[:, 2 * mp:2 * mp + 2, cols],
                                start=(mp == 0), stop=(mp == NPAIR - 1),
                                perf_mode=DR)

                def ctx_evict(qt, nsub):
                    sub = NB // nsub
                    for cc in range(2):
                        rows = slice(cc * P, (cc + 1) * P)
                        for si in range(nsub):
                            c0 = qt * NB + si * sub
                            cols = slice(c0, c0 + sub)
                            pcols = slice(si * sub, (si + 1) * sub)
                            t = evict.tile([P, sub], F32, tag="t", name="t")
                            nc.vector.tensor_mul(t[:], ctx_t[(qt, cc)][:, pcols],
                                                 recip[:, cols])
                            o = evict.tile([P, sub], F32, tag="o", name="o")
                            # bv is folded into x_bf host-side; residual-add
                            # on GpSimd (idle here), alternating with the DVE
                            # on the last quarter for a fast drain
                            eng_o = nc.gpsimd if (qt < NLN - 1 or si % 2 == 0) \
                                else nc.vector
                            eng_o.tensor_add(o[:], x_bf[:, cc, cols], t[:])
                            eng = store_eng[(qt * 2 + cc + si) % 3]
                            eng.dma_start(out=out_d[rows, cols], in_=o[:])

                # quarter 0: run key-pairs 0..6 first so the PE never stalls
                # on the last chunk's exp; pair 7 sweeps after
                den_mms(0, 0, NPAIR - 1)
                ctx_mms(0, 0, NPAIR - 1)
                den_mms(0, NPAIR - 1, NPAIR)
                ctx_mms(0, NPAIR - 1, NPAIR)
                ctx_evict(0, 1)
                for qt in range(1, NLN):
                    den_mms(qt, 0, NPAIR)
                    ctx_mms(qt, 0, NPAIR)
                    ctx_evict(qt, 1 if qt < NLN - 1 else 2)

    nc.compile()
    return nc


def get_compiled():
    global _COMPILED
    if _COMPILED is None:
        _COMPILED = build_nc()
    return _COMPILED


def _shuffle(a):
    """[2*P, N...] -> [P, 2, N...] partition-major."""
    return np.ascontiguousarray(a.reshape(2, P, *a.shape[1:]).transpose(1, 0, *range(2, a.ndim + 1)))


def make_in_maps(inputs):
    x = np.ascontiguousarray(np.asarray(inputs["x"], dtype=np.float32))
    Wq = np.asarray(inputs["Wq"], np.float32)
    Wk = np.asarray(inputs["Wk"], np.float32)
    Wv = np.asarray(inputs["Wv"], np.float32)
    M = Wq.T @ Wk                                   # scores = x^T M x
    u = SCALE * (Wk.T @ np.asarray(inputs["bq"], np.float32))
    u8 = np.zeros((C, 16), ml_dtypes.float8_e4m3)
    u8[:, 0] = (256.0 * u).astype(ml_dtypes.float8_e4m3)
    mt8 = _shuffle(np.ascontiguousarray((16.0 * M).T).astype(ml_dtypes.float8_e4m3))
    wvt8 = _shuffle(np.ascontiguousarray((16.0 * Wv).T).astype(ml_dtypes.float8_e4m3))
    u8s = _shuffle(u8)
    bvs = _shuffle(np.asarray(inputs["bv"], np.float32).reshape(C, 1))
    wb = np.concatenate([
        mt8.reshape(P, 512).view(np.uint8),
        wvt8.reshape(P, 512).view(np.uint8),
        u8s.reshape(P, 32).view(np.uint8),
        np.ascontiguousarray(bvs.reshape(P, 2)).view(np.uint8),
    ], axis=1)
    assert wb.shape == (P, WBLOB)
    bvc = np.asarray(inputs["bv"], np.float32).reshape(C, 1)
    return [{"xbf": _shuffle((x[i] + bvc).astype(ml_dtypes.bfloat16)),
             "x8": _shuffle(x[i].astype(ml_dtypes.float8_e4m3)),
             "wb": wb} for i in range(B)]


def run(inputs, trace=False, **kwargs):
    nc = get_compiled()
    res = run_bass_kernel_spmd(nc, make_in_maps(inputs),
                               core_ids=list(range(B)), trace=trace, **kwargs)
    out = np.stack([res.results[i]["out"] for i in range(B)], axis=0)
    return out.astype(np.float32), res


def kernel(**inputs):
    out, _ = run(inputs)
    return out
